# revision 4
# baseline (speedup 1.0000x reference)
"""Llama GQA attention layer (B=1, S=2048, E=4096, H=32, HKV=8, D=128) on 8
Trainium2 NeuronCores.

Sharding: tensor-parallel over heads. Core c owns Q heads 4c..4c+3 and KV head
c (KV groups stay intact), plus the matching Wo input-dim slice. Each core
computes a full [S, E] partial of the o_proj output in bf16; the host sums the
8 partials (the "all-reduce after o_proj").

All matmuls run in bf16 (1 cyc/row on the PE at 512-wide moving dim, with
automatic fast-weight-load; fp32r streams at ~1.3 cyc/row and pays 225ns
weight loads). PSUM accumulation stays fp32. l2 error budget is 2e-2; bf16
rounding of inputs/weights/probabilities lands well under 1e-2.

Per-core dataflow:
  phase A (PE-dense): per token group g (512 tokens):
    qT/kT/vT = W @ hs.T    6 psum chains x 32 E-chunks, [feat, tok] layout.
    Wq|Wk|Wv are host-packed into one [E, 768] tensor so each E-chunk is a
    single contiguous DMA and the first matmul fires ~2us after launch.
    RoPE off-PE: psum -> sbuf copy (scalar), half-swap via SBUF->SBUF DMA,
    cos/sin muls (DVE) -> qro/krope bf16. v: psum -> bf16 sbuf (scalar),
    PE-transposed to vnat [tok, d] between later groups' QKV streams.
  phase B attention, per query group G (causal: key tiles ki <= 4G+3, with
  moving-dim trimming + triangular mask add on diagonal tiles):
    scoresT[k, q] = krope_tile^T @ qro  (PSUM), exp on ScalarE -> bf16 expT
    avT[d, q]  accumulated over ki on PE (vnat stationary)
    den[h]     accumulated over ki on PE (ones[128,1] stationary, ~free
               weight load); all 4 heads share one psum bank at partition
               offsets 0/32/64/96.
    epilogue per head: DVE reciprocal on the [1, 512] den row only (full-tile
    reciprocal costs ~12 cyc/elem), K=1 ones matmul broadcasts it to 128
    partitions, DVE mul -> aoT bf16 (kept in SBUF, no DRAM spill).
  phase C o_proj: out[t, e] = sum_h aoT[:, h-tile]^T @ woT[h], 3 psum
    banks rotating, drains alternate scalar/vector, bf16 partials to DRAM.
"""

import sys
import types

if "/opt/trn_rl_repo" not in sys.path:
    sys.path.insert(0, "/opt/trn_rl_repo")

import numpy as np
import ml_dtypes

import concourse.bass as bass
import concourse.tile as tile
from concourse import bacc, mybir
from concourse.bass_utils import run_bass_kernel_spmd
from concourse.masks import make_identity

F32 = mybir.dt.float32
BF16 = mybir.dt.bfloat16
EXP = mybir.ActivationFunctionType.Exp
NPBF = ml_dtypes.bfloat16

S = 2048
E = 4096
H = 32
HKV = 8
D = 128
NCORES = 8
HL = H // NCORES          # 4 local q heads per core
TG = 512                  # token group (moving-dim tile)
NG = S // TG              # 4 token groups
NE = E // 128             # 32 contraction chunks
NK = S // 128             # 16 key tiles
FQKV = HL * D + 2 * D     # 768 packed output features per core
NEG = -1e9

TRACE = [False]
LAST_EXEC_NS = [None]
LAST_RES = [None]

_PROGRAMS = {}


def _install_ntff_hook():
    if "antenv.axon_hooks" in sys.modules:
        return
    mod = types.ModuleType("antenv.axon_hooks")
    hook = [None]
    mod.set_axon_ntff_profile_hook = lambda h: hook.__setitem__(0, h)
    mod.get_axon_ntff_profile_hook = lambda: hook[0]
    sys.modules["antenv.axon_hooks"] = mod
    try:
        from trn_agent_boot.trn_boot import _ntff_profile_via_ctypes

        mod.set_axon_ntff_profile_hook(
            _ntff_profile_via_ctypes("/opt/axon/libaxon_pjrt.so"))
    except Exception:
        pass


def set_trace(on=True):
    if on:
        _install_ntff_hook()
    TRACE[0] = on


def _build_program(mode):
    """mode: 'causal' (skip above-diagonal key tiles, trim + triangular mask
    on diagonal tiles), 'full' (no mask), 'general' (additive mask streamed
    from DRAM)."""
    nc = bacc.Bacc(trn_type="TRN2", target_bir_lowering=False, debug=False)

    # group-major hsT: [g, E, TG] so each [128, TG] chunk is contiguous
    hsT_d = nc.dram_tensor("hsT", [NG, E, TG], BF16, kind="ExternalInput").ap()
    # packed [Wq | Wk | Wv] transposed: rows are E, cols 768
    wqkv_d = nc.dram_tensor("wqkvT", [E, FQKV], BF16, kind="ExternalInput").ap()
    woT_d = nc.dram_tensor("woT", [HL * D, E], BF16, kind="ExternalInput").ap()
    cos_d = nc.dram_tensor("cosT", [D, S], F32, kind="ExternalInput").ap()
    sin_d = nc.dram_tensor("sinT", [D, S], F32, kind="ExternalInput").ap()
    if mode == "causal":
        cmask_d = nc.dram_tensor("cmask", [128, 128], F32,
                                 kind="ExternalInput").ap()
    elif mode == "general":
        maskT_d = nc.dram_tensor("maskT", [S, S], F32, kind="ExternalInput").ap()
    # tile-major output: [ti, eg, 128, TG] so each store is contiguous
    outp_d = nc.dram_tensor("outp", [NK, E // TG, 128, TG], BF16,
                            kind="ExternalOutput").ap()

    with tile.TileContext(nc) as tc:
        with tc.tile_pool(name="const", bufs=1) as cpool, \
             tc.tile_pool(name="persist", bufs=1) as pp, \
             tc.tile_pool(name="wqkv", bufs=1) as wp, \
             tc.tile_pool(name="cs", bufs=1) as csp, \
             tc.tile_pool(name="hst", bufs=8) as hp, \
             tc.tile_pool(name="rope", bufs=1) as rp, \
             tc.tile_pool(name="attn", bufs=1) as ap_, \
             tc.tile_pool(name="outb", bufs=1) as obp, \
             tc.tile_pool(name="ps", bufs=1, space="PSUM") as ps:

            # ---- constants ----
            identf = cpool.tile([128, 128], F32)
            make_identity(nc, identf)
            ident = cpool.tile([128, 128], BF16)
            nc.vector.tensor_copy(ident, identf)
            onesf = cpool.tile([128, 1], F32)
            nc.vector.memset(onesf, 1.0)
            ones_col = cpool.tile([128, 1], BF16)
            nc.vector.tensor_copy(ones_col, onesf)
            onesrf = cpool.tile([1, 128], F32)
            nc.vector.memset(onesrf, 1.0)
            ones_row = cpool.tile([1, 128], BF16)
            nc.vector.tensor_copy(ones_row, onesrf)
            if mode == "causal":
                cmask = cpool.tile([128, 128], F32)
                nc.sync.dma_start(out=cmask, in_=cmask_d)

            # ---- persistent activations ----
            krope = pp.tile([128, S], BF16)               # [d, tok]
            vnat = pp.tile([128, NK, 128], BF16)          # [tok%128, ktile, d]
            ao = pp.tile([128, HL, S], BF16)              # [d, head, tok]
            qro = pp.tile([128, NG, HL, TG], BF16)        # [d, g, head, tok]

            # ---- weights: chunk loads interleaved with group-0 hsT so the
            # first QKV matmul fires almost immediately ----
            w_sb = wp.tile([128, NE, FQKV], BF16)
            wo_sb = wp.tile([128, HL, E], BF16)
            cos_sb = csp.tile([128, S], F32)
            sin_sb = csp.tile([128, S], F32)

            wqkv_r = wqkv_d.rearrange("(ne p) f -> p ne f", p=128)
            hst0 = []
            for e in range(NE):
                nc.sync.dma_start(out=w_sb[:, e, :], in_=wqkv_r[:, e, :])
                hst = hp.tile([128, TG], BF16, tag="hst")
                nc.sync.dma_start(
                    out=hst, in_=hsT_d[0, 128 * e:128 * (e + 1), :])
                hst0.append(hst)
                if e == 2:
                    nc.scalar.dma_start(out=cos_sb, in_=cos_d)
                    nc.scalar.dma_start(out=sin_sb, in_=sin_d)

            # ================= phase A: QKV projection + RoPE =================
            def emit_qkv(g):
                q_ps = [ps.tile([128, TG], F32, tag=f"A{f}", name=f"q_ps{f}")
                        for f in range(HL)]
                k_ps = ps.tile([128, TG], F32, tag="A4", name="k_ps")
                v_ps = ps.tile([128, TG], F32, tag="A5", name="v_ps")
                for e in range(NE):
                    if g == 0:
                        hst = hst0[e]
                    else:
                        hst = hp.tile([128, TG], BF16, tag="hst")
                        nc.sync.dma_start(
                            out=hst, in_=hsT_d[g, 128 * e:128 * (e + 1), :])
                    st, sp = (e == 0), (e == NE - 1)
                    for f in range(HL):
                        nc.tensor.matmul(
                            q_ps[f], w_sb[:, e, 128 * f:128 * (f + 1)],
                            hst, start=st, stop=sp)
                    nc.tensor.matmul(k_ps, w_sb[:, e, 512:640], hst,
                                     start=st, stop=sp)
                    nc.tensor.matmul(v_ps, w_sb[:, e, 640:768], hst,
                                     start=st, stop=sp)
                return q_ps, k_ps, v_ps

            def emit_rope(g, q_ps, k_ps, v_ps):
                t0 = g * TG
                cs = cos_sb[:, t0:t0 + TG]
                sn = sin_sb[:, t0:t0 + TG]
                pairs = [(q_ps[f], qro[:, g, f, :]) for f in range(HL)]
                pairs.append((k_ps, krope[:, t0:t0 + TG]))
                for x_ps, out_ap in pairs:
                    xs = rp.tile([128, TG], F32, tag="xs", bufs=3)
                    nc.scalar.copy(out=xs, in_=x_ps)
                    swp = rp.tile([128, TG], F32, tag="swp", bufs=3)
                    nc.sync.dma_start(out=swp[0:64, :], in_=xs[64:128, :])
                    nc.sync.dma_start(out=swp[64:128, :], in_=xs[0:64, :])
                    p1 = rp.tile([128, TG], F32, tag="p1", bufs=2)
                    nc.vector.tensor_mul(p1, x_ps, cs)
                    nc.vector.tensor_mul(swp, swp, sn)
                    nc.vector.tensor_add(out_ap, p1, swp)
                vs = rp.tile([128, TG], BF16, tag="vs", bufs=2)
                nc.scalar.copy(out=vs, in_=v_ps)
                return vs

            def emit_vtr(g, vs):
                for j in range(4):
                    tr = ps.tile([128, 128], BF16, tag="A6", name="tr_ps")
                    nc.tensor.transpose(tr, vs[:, 128 * j:128 * (j + 1)], ident)
                    nc.vector.tensor_copy(vnat[:, 4 * g + j, :], tr)

            vs_pend = []
            for g in range(NG):
                qkv = emit_qkv(g)
                if vs_pend:
                    emit_vtr(*vs_pend.pop())
                vs = emit_rope(g, *qkv)
                vs_pend.append((g, vs))
            emit_vtr(*vs_pend.pop())

            # wo loads run on the DMA engines during attention
            woT_r = woT_d.rearrange("(h p) e -> p h e", p=128)
            for eg in range(E // TG):
                nc.sync.dma_start(
                    out=wo_sb[:, :, TG * eg:TG * (eg + 1)],
                    in_=woT_r[:, :, TG * eg:TG * (eg + 1)])

            # ================= phase B: attention =================
            def emit_attn(G):
                nk = 4 * G + 4 if mode == "causal" else NK
                av = [ps.tile([128, TG], F32, tag=f"A{h}", name=f"av{h}")
                      for h in range(HL)]
                # softmax denominators: 2 heads per psum bank, one row each
                # at partitions 0/32 (32-aligned for matmul tile_position)
                den2 = [ps.tile([128, TG], F32, tag=t, name="den2")
                        for t in ("A6", "A7")]
                pend = []

                def drain_av(item):
                    ki, h, c0, ex = item
                    nc.tensor.matmul(av[h][:, c0:], vnat[:, ki, :], ex[:, c0:],
                                     start=(ki == 0), stop=(ki == nk - 1),
                                     skip_group_check=True)
                    dn = den2[h // 2]
                    r0 = 32 * (h % 2)
                    nc.tensor.matmul(dn[r0:r0 + 1, c0:], ones_col, ex[:, c0:],
                                     start=(ki == 0), stop=(ki == nk - 1),
                                     skip_group_check=True)

                for ki in range(nk):
                    c0 = max(0, 128 * ki - TG * G) if mode == "causal" else 0
                    kk = krope[:, 128 * ki:128 * (ki + 1)]
                    for h in range(HL):
                        s = ps.tile([128, TG], F32, name="s_ps",
                                    tag=["A4", "A5"][(ki * HL + h) % 2])
                        nc.tensor.matmul(s[:, c0:], kk, qro[:, G, h, c0:],
                                         start=True, stop=True)
                        if mode == "causal" and ki >= 4 * G:
                            nc.vector.tensor_add(s[:, c0:c0 + 128],
                                                 s[:, c0:c0 + 128], cmask)
                        elif mode == "general":
                            mt = ap_.tile([128, TG], F32, tag="mt", bufs=4)
                            nc.sync.dma_start(
                                out=mt, in_=maskT_d[128 * ki:128 * (ki + 1),
                                                    TG * G:TG * (G + 1)])
                            nc.vector.tensor_add(s, s, mt)
                        ex = ap_.tile([128, TG], BF16, tag="ex", bufs=8)
                        nc.scalar.activation(out=ex[:, c0:], in_=s[:, c0:],
                                             func=EXP)
                        pend.append((ki, h, c0, ex))
                        # interleave drains between scores to keep the exp
                        # pipeline fed without stalling on s-bank reuse
                        while len(pend) > 6:
                            drain_av(pend.pop(0))
                while pend:
                    drain_av(pend.pop(0))
                t0 = G * TG
                for h in range(HL):
                    r0 = 32 * (h % 2)
                    rc = ap_.tile([1, TG], BF16, tag="rc", bufs=2)
                    with nc.allow_low_precision(reason="softmax recip"):
                        nc.vector.reciprocal(rc, den2[h // 2][r0:r0 + 1, :])
                    bc = ps.tile([128, TG], F32, name="bc_ps",
                                 tag=["A4", "A5"][h % 2])
                    nc.tensor.matmul(bc, ones_row, rc, start=True, stop=True)
                    bcs = ap_.tile([128, TG], BF16, tag="bcs", bufs=2)
                    nc.vector.tensor_copy(bcs, bc)
                    nc.vector.tensor_mul(ao[:, h, t0:t0 + TG], av[h], bcs)

            for G in range(NG):
                emit_attn(G)

            # ================= phase C: o_proj =================
            for ti in range(NK):
                for eg in range(E // TG):
                    o_ps = ps.tile([128, TG], F32, name="o_ps",
                                   tag=["A0", "A1", "A2"][(ti * 8 + eg) % 3])
                    for h in range(HL):
                        nc.tensor.matmul(
                            o_ps, ao[:, h, 128 * ti:128 * (ti + 1)],
                            wo_sb[:, h, TG * eg:TG * (eg + 1)],
                            start=(h == 0), stop=(h == HL - 1))
                    ob = obp.tile([128, TG], BF16, tag="ob", bufs=4)
                    if eg % 2 == 0:
                        nc.scalar.copy(out=ob, in_=o_ps)
                    else:
                        nc.vector.tensor_copy(ob, o_ps)
                    nc.sync.dma_start(out=outp_d[ti, eg], in_=ob)

    nc.compile()
    return nc


_CAUSAL_MASK_TILES = None


def _causal_mask_tiles():
    global _CAUSAL_MASK_TILES
    if _CAUSAL_MASK_TILES is None:
        kp = np.arange(128)[:, None]
        qc = np.arange(128)[None, :]
        _CAUSAL_MASK_TILES = np.where(qc >= kp, 0.0, NEG).astype(np.float32)
    return _CAUSAL_MASK_TILES


def _rope_tables(position_ids):
    pos = np.asarray(position_ids[0]).astype(np.float32)          # [S]
    inv_freq = (1.0 / (10000.0 ** (np.arange(0, D, 2, dtype=np.float32) / D)))
    freqs = pos[:, None] * inv_freq[None, :]                      # [S, 64]
    emb = np.concatenate([freqs, freqs], axis=1)                  # [S, 128]
    cosT = np.cos(emb).T.astype(np.float32)                       # [128, S]
    sinT = np.sin(emb).T.astype(np.float32)
    sinflipT = np.concatenate([-sinT[:64], sinT[64:]], axis=0).astype(np.float32)
    return np.ascontiguousarray(cosT), np.ascontiguousarray(sinflipT)


def kernel(hidden_states, position_ids, attention_mask, Wq, Wk, Wv, Wo):
    hidden_states = np.asarray(hidden_states)
    B = hidden_states.shape[0]
    assert hidden_states.shape == (B, S, E), hidden_states.shape
    assert B == 1

    mask = np.asarray(attention_mask, dtype=np.float32)[0, 0]
    if not mask.any():
        mode = "full"
    elif np.array_equal(mask, np.triu(np.full((S, S), NEG, dtype=np.float32), 1)):
        mode = "causal"
    else:
        mode = "general"

    if mode not in _PROGRAMS:
        _PROGRAMS[mode] = _build_program(mode)
    nc = _PROGRAMS[mode]

    hs = np.asarray(hidden_states[0], dtype=np.float32)
    # [E, S] -> group-major [NG, E, TG], bf16
    hsT = np.ascontiguousarray(
        hs.T.reshape(E, NG, TG).transpose(1, 0, 2)).astype(NPBF)
    cosT, sinflipT = _rope_tables(np.asarray(position_ids))
    # fold the 1/sqrt(D) score scaling into Wq so q and k share rope tables
    Wq = np.asarray(Wq, dtype=np.float32) * np.float32(1.0 / np.sqrt(D))
    Wk = np.asarray(Wk, dtype=np.float32)
    Wv = np.asarray(Wv, dtype=np.float32)
    Wo = np.asarray(Wo, dtype=np.float32)

    in_maps = []
    for c in range(NCORES):
        wqkv = np.concatenate([
            Wq[512 * c:512 * (c + 1), :].T,
            Wk[128 * c:128 * (c + 1), :].T,
            Wv[128 * c:128 * (c + 1), :].T,
        ], axis=1)
        m = {
            "hsT": hsT,
            "wqkvT": np.ascontiguousarray(wqkv).astype(NPBF),
            "woT": np.ascontiguousarray(Wo[:, 512 * c:512 * (c + 1)].T).astype(NPBF),
            "cosT": cosT, "sinT": sinflipT,
        }
        if mode == "causal":
            m["cmask"] = _causal_mask_tiles()
        elif mode == "general":
            m["maskT"] = np.ascontiguousarray(mask.T)
        in_maps.append(m)

    res = run_bass_kernel_spmd(nc, in_maps, core_ids=list(range(NCORES)),
                               trace=TRACE[0])
    LAST_EXEC_NS[0] = res.exec_time_ns
    LAST_RES[0] = res

    acc = np.zeros((NK, E // TG, 128, TG), dtype=np.float32)
    for c in range(NCORES):
        acc += res.results[c]["outp"].astype(np.float32)
    out = acc.transpose(0, 2, 1, 3).reshape(S, E)
    return out[None, :, :]


# revision 12
# speedup vs baseline: 1.0043x; 1.0043x over previous
"""Llama GQA attention layer (B=1, S=2048, E=4096, H=32, HKV=8, D=128) on 8
Trainium2 NeuronCores.

Sharding: tensor-parallel over heads. Core c owns Q heads 4c..4c+3 and KV head
c (KV groups stay intact), plus the matching Wo input-dim slice. Each core
computes a full [S, E] partial of the o_proj output in bf16; the host sums the
8 partials (the "all-reduce after o_proj").

All matmuls run in bf16 (1 cyc/row on the PE at 512-wide moving dim, with
automatic fast-weight-load; fp32r streams at ~1.3 cyc/row and pays 225ns
weight loads). PSUM accumulation stays fp32. l2 error budget is 2e-2; bf16
rounding of inputs/weights/probabilities lands well under 1e-2.

Per-core dataflow:
  phase A (PE-dense): per token group g (512 tokens):
    qT/kT/vT = W @ hs.T    6 psum chains x 32 E-chunks, [feat, tok] layout.
    Wq|Wk|Wv are host-packed into one [E, 768] tensor so each E-chunk is a
    single contiguous DMA and the first matmul fires ~2us after launch.
    RoPE off-PE: psum -> sbuf copy (scalar), half-swap via SBUF->SBUF DMA,
    cos/sin muls (DVE) -> qro/krope bf16. v: psum -> bf16 sbuf (scalar),
    PE-transposed to vnat [tok, d] between later groups' QKV streams.
  phase B attention, per query group G (causal: key tiles ki <= 4G+3, with
  moving-dim trimming + triangular mask add on diagonal tiles):
    scoresT[k, q] = krope_tile^T @ qro  (PSUM), exp on ScalarE -> bf16 expT
    avT[d, q]  accumulated over ki on PE (vnat stationary)
    den[h]     accumulated over ki on PE (ones[128,1] stationary, ~free
               weight load); all 4 heads share one psum bank at partition
               offsets 0/32/64/96.
    epilogue per head: DVE reciprocal on the [1, 512] den row only (full-tile
    reciprocal costs ~12 cyc/elem), K=1 ones matmul broadcasts it to 128
    partitions, DVE mul -> aoT bf16 (kept in SBUF, no DRAM spill).
  phase C o_proj: out[t, e] = sum_h aoT[:, h-tile]^T @ woT[h], 3 psum
    banks rotating, drains alternate scalar/vector, bf16 partials to DRAM.
"""

import sys
import types

if "/opt/trn_rl_repo" not in sys.path:
    sys.path.insert(0, "/opt/trn_rl_repo")

import numpy as np
import ml_dtypes

import concourse.bass as bass
import concourse.tile as tile
from concourse import bacc, mybir
from concourse.bass_utils import run_bass_kernel_spmd
from concourse.masks import make_identity

F32 = mybir.dt.float32
BF16 = mybir.dt.bfloat16
EXP = mybir.ActivationFunctionType.Exp
NPBF = ml_dtypes.bfloat16

S = 2048
E = 4096
H = 32
HKV = 8
D = 128
NCORES = 8
HL = H // NCORES          # 4 local q heads per core
TG = 512                  # token group (moving-dim tile)
NG = S // TG              # 4 token groups
NE = E // 128             # 32 contraction chunks
NK = S // 128             # 16 key tiles
FQKV = HL * D + 2 * D     # 768 packed output features per core
NEG = -1e9

TRACE = [False]
LAST_EXEC_NS = [None]
LAST_RES = [None]

_PROGRAMS = {}


def _install_ntff_hook():
    if "antenv.axon_hooks" in sys.modules:
        return
    mod = types.ModuleType("antenv.axon_hooks")
    hook = [None]
    mod.set_axon_ntff_profile_hook = lambda h: hook.__setitem__(0, h)
    mod.get_axon_ntff_profile_hook = lambda: hook[0]
    sys.modules["antenv.axon_hooks"] = mod
    try:
        from trn_agent_boot.trn_boot import _ntff_profile_via_ctypes

        mod.set_axon_ntff_profile_hook(
            _ntff_profile_via_ctypes("/opt/axon/libaxon_pjrt.so"))
    except Exception:
        pass


def set_trace(on=True):
    if on:
        _install_ntff_hook()
    TRACE[0] = on


def _build_program(mode):
    """mode: 'causal' (skip above-diagonal key tiles, trim + triangular mask
    on diagonal tiles), 'full' (no mask), 'general' (additive mask streamed
    from DRAM)."""
    nc = bacc.Bacc(trn_type="TRN2", target_bir_lowering=False, debug=False)

    # group-major hsT: [g, E, TG] so each [128, TG] chunk is contiguous
    hsT_d = nc.dram_tensor("hsT", [NG, E, TG], BF16, kind="ExternalInput").ap()
    # packed [Wq | Wk | Wv] transposed: rows are E, cols 768
    wqkv_d = nc.dram_tensor("wqkvT", [E, FQKV], BF16, kind="ExternalInput").ap()
    woT_d = nc.dram_tensor("woT", [HL * D, E], BF16, kind="ExternalInput").ap()
    cos_d = nc.dram_tensor("cosT", [D, S], F32, kind="ExternalInput").ap()
    sin_d = nc.dram_tensor("sinT", [D, S], F32, kind="ExternalInput").ap()
    if mode == "causal":
        cmask_d = nc.dram_tensor("cmask", [128, 128], F32,
                                 kind="ExternalInput").ap()
    elif mode == "general":
        maskT_d = nc.dram_tensor("maskT", [S, S], F32, kind="ExternalInput").ap()
    # tile-major output: [ti, eg, 128, TG] so each store is contiguous
    outp_d = nc.dram_tensor("outp", [NK, E // TG, 128, TG], BF16,
                            kind="ExternalOutput").ap()

    with tile.TileContext(nc) as tc:
        with tc.tile_pool(name="const", bufs=1) as cpool, \
             tc.tile_pool(name="persist", bufs=1) as pp, \
             tc.tile_pool(name="wqkv", bufs=1) as wp, \
             tc.tile_pool(name="cs", bufs=1) as csp, \
             tc.tile_pool(name="hst", bufs=8) as hp, \
             tc.tile_pool(name="rope", bufs=1) as rp, \
             tc.tile_pool(name="attn", bufs=1) as ap_, \
             tc.tile_pool(name="outb", bufs=1) as obp, \
             tc.tile_pool(name="ps", bufs=1, space="PSUM") as ps:

            # ---- constants ----
            identf = cpool.tile([128, 128], F32)
            make_identity(nc, identf)
            ident = cpool.tile([128, 128], BF16)
            nc.vector.tensor_copy(ident, identf)
            onesf = cpool.tile([128, 1], F32)
            nc.vector.memset(onesf, 1.0)
            ones_col = cpool.tile([128, 1], BF16)
            nc.vector.tensor_copy(ones_col, onesf)
            # ones rows at partition bases 0 and 32 (matmul operands must
            # share a 32-aligned base partition with the den rows they read)
            onesrf = cpool.tile([64, 128], F32)
            nc.vector.memset(onesrf, 1.0)
            ones_rows = cpool.tile([64, 128], BF16)
            nc.vector.tensor_copy(ones_rows, onesrf)
            if mode == "causal":
                cmask = cpool.tile([128, 128], F32)
                nc.sync.dma_start(out=cmask, in_=cmask_d)

            # ---- persistent activations ----
            krope = pp.tile([128, S], BF16)               # [d, tok]
            vnat = pp.tile([128, NK, 128], BF16)          # [tok%128, ktile, d]
            ao = pp.tile([128, HL, S], BF16)              # [d, head, tok]
            qro = pp.tile([128, NG, HL, TG], BF16)        # [d, g, head, tok]

            # ---- weights: chunk loads interleaved with group-0 hsT so the
            # first QKV matmul fires almost immediately ----
            w_sb = wp.tile([128, NE, FQKV], BF16)
            wo_sb = wp.tile([128, HL, E], BF16)
            cos_sb = csp.tile([128, S], F32)
            sin_sb = csp.tile([128, S], F32)

            # weight chunks batched x4 on the scalar queue, group-0 hsT on the
            # sync queue: both pipelines issue in parallel so the first
            # matmul fires as early as possible (each dma_start costs ~0.6us
            # of descriptor generation on its issuing queue)
            wqkv_r = wqkv_d.rearrange("(ne p) f -> p ne f", p=128)
            hst0 = []
            for e in range(NE):
                if e % 4 == 0:
                    nc.scalar.dma_start(out=w_sb[:, e:e + 4, :],
                                        in_=wqkv_r[:, e:e + 4, :])
                hst = hp.tile([128, TG], BF16, tag="hst")
                nc.sync.dma_start(
                    out=hst, in_=hsT_d[0, 128 * e:128 * (e + 1), :])
                hst0.append(hst)
                if e == 2:
                    nc.scalar.dma_start(out=cos_sb, in_=cos_d)
                    nc.scalar.dma_start(out=sin_sb, in_=sin_d)

            # ================= phase A: QKV projection + RoPE =================
            def emit_qkv(g):
                q_ps = [ps.tile([128, TG], F32, tag=f"A{f}", name=f"q_ps{f}")
                        for f in range(HL)]
                k_ps = ps.tile([128, TG], F32, tag="A4", name="k_ps")
                v_ps = ps.tile([128, TG], F32, tag="A5", name="v_ps")
                for e in range(NE):
                    if g == 0:
                        hst = hst0[e]
                    else:
                        hst = hp.tile([128, TG], BF16, tag="hst")
                        nc.sync.dma_start(
                            out=hst, in_=hsT_d[g, 128 * e:128 * (e + 1), :])
                    st, sp = (e == 0), (e == NE - 1)
                    for f in range(HL):
                        nc.tensor.matmul(
                            q_ps[f], w_sb[:, e, 128 * f:128 * (f + 1)],
                            hst, start=st, stop=sp)
                    nc.tensor.matmul(k_ps, w_sb[:, e, 512:640], hst,
                                     start=st, stop=sp)
                    nc.tensor.matmul(v_ps, w_sb[:, e, 640:768], hst,
                                     start=st, stop=sp)
                return q_ps, k_ps, v_ps

            def emit_rope(g, q_ps, k_ps, v_ps):
                t0 = g * TG
                cs = cos_sb[:, t0:t0 + TG]
                sn = sin_sb[:, t0:t0 + TG]
                pairs = [(q_ps[f], qro[:, g, f, :]) for f in range(HL)]
                kpair = (k_ps, krope[:, t0:t0 + TG])
                # last group: drain k first so attention's first score matmul
                # (s-bank reuse of the k psum tag) unblocks earliest
                pairs = [kpair] + pairs if g == NG - 1 else pairs + [kpair]
                for x_ps, out_ap in pairs:
                    xs = rp.tile([128, TG], F32, tag="xs", bufs=3)
                    nc.scalar.copy(out=xs, in_=x_ps)
                    swp = rp.tile([128, TG], F32, tag="swp", bufs=3)
                    nc.gpsimd.dma_start(out=swp[0:64, :], in_=xs[64:128, :])
                    nc.gpsimd.dma_start(out=swp[64:128, :], in_=xs[0:64, :])
                    p1 = rp.tile([128, TG], F32, tag="p1", bufs=2)
                    nc.vector.tensor_mul(p1, x_ps, cs)
                    nc.vector.tensor_mul(swp, swp, sn)
                    nc.vector.tensor_add(out_ap, p1, swp)
                vs = rp.tile([128, TG], BF16, tag="vs", bufs=2)
                nc.scalar.copy(out=vs, in_=v_ps)
                return vs

            def emit_vtr(g, vs):
                for j in range(4):
                    tr = ps.tile([128, 128], BF16, tag="A6", name="tr_ps")
                    nc.tensor.transpose(tr, vs[:, 128 * j:128 * (j + 1)], ident)
                    nc.vector.tensor_copy(vnat[:, 4 * g + j, :], tr)

            vs_pend = []
            for g in range(NG):
                qkv = emit_qkv(g)
                if vs_pend:
                    emit_vtr(*vs_pend.pop())
                vs = emit_rope(g, *qkv)
                vs_pend.append((g, vs))
            emit_vtr(*vs_pend.pop())

            # wo loads issue from the (otherwise idle) gpsimd queue during
            # attention
            woT_r = woT_d.rearrange("(h p) e -> p h e", p=128)
            for eg in range(E // TG):
                nc.gpsimd.dma_start(
                    out=wo_sb[:, :, TG * eg:TG * (eg + 1)],
                    in_=woT_r[:, :, TG * eg:TG * (eg + 1)])

            # ================= phase B: attention =================
            # Each group's softmax epilogue (reciprocal -> broadcast matmul ->
            # ao multiply) is EMITTED into the next group's instruction stream
            # so its latency chains never stall the PE.
            def emit_attn(G, prev_epi):
                nk = 4 * G + 4 if mode == "causal" else NK
                av = [ps.tile([128, TG], F32, tag=f"A{h}", name=f"av{h}")
                      for h in range(HL)]
                # softmax denominators: 2 heads per psum bank, one row each
                # at partitions 0/32 (32-aligned for matmul tile_position)
                den2 = [ps.tile([128, TG], F32, tag=t, name="den2")
                        for t in ("A6", "A7")]
                pend = []

                def drain_av(item):
                    ki, h, c0, ex = item
                    nc.tensor.matmul(av[h][:, c0:], vnat[:, ki, :], ex[:, c0:],
                                     start=(ki == 0), stop=(ki == nk - 1),
                                     skip_group_check=True)
                    dn = den2[h // 2]
                    r0 = 32 * (h % 2)
                    nc.tensor.matmul(dn[r0:r0 + 1, c0:], ones_col, ex[:, c0:],
                                     start=(ki == 0), stop=(ki == nk - 1),
                                     skip_group_check=True)

                for ki in range(nk):
                    c0 = max(0, 128 * ki - TG * G) if mode == "causal" else 0
                    kk = krope[:, 128 * ki:128 * (ki + 1)]
                    for h in range(HL):
                        s = ps.tile([128, TG], F32, name="s_ps",
                                    tag=["A4", "A5"][(ki * HL + h) % 2])
                        nc.tensor.matmul(s[:, c0:], kk, qro[:, G, h, c0:],
                                         start=True, stop=True)
                        if mode == "causal" and ki >= 4 * G:
                            nc.vector.tensor_add(s[:, c0:c0 + 128],
                                                 s[:, c0:c0 + 128], cmask)
                        elif mode == "general":
                            mt = ap_.tile([128, TG], F32, tag="mt", bufs=4)
                            nc.sync.dma_start(
                                out=mt, in_=maskT_d[128 * ki:128 * (ki + 1),
                                                    TG * G:TG * (G + 1)])
                            nc.vector.tensor_add(s, s, mt)
                        ex = ap_.tile([128, TG], BF16, tag="ex", bufs=8)
                        nc.scalar.activation(out=ex[:, c0:], in_=s[:, c0:],
                                             func=EXP)
                        pend.append((ki, h, c0, ex))
                        # previous group's deferred epilogue slots in here,
                        # long after its reciprocal chain resolved
                        if prev_epi:
                            prev_epi.pop(0)()
                        # interleave drains between scores to keep the exp
                        # pipeline fed without stalling on s-bank reuse
                        while len(pend) > 6:
                            drain_av(pend.pop(0))
                while pend:
                    drain_av(pend.pop(0))
                while prev_epi:
                    prev_epi.pop(0)()

                t0 = G * TG
                epi = []
                for h in range(HL):
                    r0 = 32 * (h % 2)
                    dnrow = den2[h // 2][r0:r0 + 1, :]
                    # off the critical path: psum -> sbuf row copy, fast
                    # approximate reciprocal (fp32 seed trick), bf16 cast
                    rcb = ap_.tile([64, TG], BF16, tag="rcb", bufs=2)
                    with nc.allow_low_precision(reason="softmax recip"):
                        nc.vector.reciprocal(rcb[r0:r0 + 1, :], dnrow)
                    # create the bc psum tile NOW so its bank-tag sequence
                    # precedes the next group's den2 tiles (emitting its
                    # matmul later would otherwise deadlock the PE queue)
                    bc = ps.tile([128, TG], F32, name="bc_ps",
                                 tag=["A6", "A7"][h % 2])

                    def mk(h=h, r0=r0, rcb=rcb, bc=bc, avh=av[h]):
                        def emit_epi():
                            nc.tensor.matmul(bc, ones_rows[r0:r0 + 1, :],
                                             rcb[r0:r0 + 1, :],
                                             start=True, stop=True)
                            bcs = ap_.tile([128, TG], BF16, tag="bcs", bufs=2)
                            nc.vector.tensor_copy(bcs, bc)
                            nc.vector.tensor_mul(ao[:, h, t0:t0 + TG], avh, bcs)
                        return emit_epi
                    epi.append(mk())
                return epi

            epi = []
            for G in range(NG):
                epi = emit_attn(G, epi)

            # ================= phase C: o_proj =================
            # group 3's deferred epilogue must fully precede the first o_proj
            # matmul: o_ps reuses the av bank tags, whose release depends on
            # the epilogue's PE-side bc matmuls
            while epi:
                epi.pop(0)()
            for ti in range(NK):
                for eg in range(E // TG):
                    o_ps = ps.tile([128, TG], F32, name="o_ps",
                                   tag=["A0", "A1", "A2"][(ti * 8 + eg) % 3])
                    for h in range(HL):
                        nc.tensor.matmul(
                            o_ps, ao[:, h, 128 * ti:128 * (ti + 1)],
                            wo_sb[:, h, TG * eg:TG * (eg + 1)],
                            start=(h == 0), stop=(h == HL - 1))
                    ob = obp.tile([128, TG], BF16, tag="ob", bufs=4)
                    if eg % 2 == 0:
                        nc.scalar.copy(out=ob, in_=o_ps)
                    else:
                        nc.vector.tensor_copy(ob, o_ps)
                    nc.gpsimd.dma_start(out=outp_d[ti, eg], in_=ob)

    nc.compile()
    return nc


_CAUSAL_MASK_TILES = None


def _causal_mask_tiles():
    global _CAUSAL_MASK_TILES
    if _CAUSAL_MASK_TILES is None:
        kp = np.arange(128)[:, None]
        qc = np.arange(128)[None, :]
        _CAUSAL_MASK_TILES = np.where(qc >= kp, 0.0, NEG).astype(np.float32)
    return _CAUSAL_MASK_TILES


def _rope_tables(position_ids):
    pos = np.asarray(position_ids[0]).astype(np.float32)          # [S]
    inv_freq = (1.0 / (10000.0 ** (np.arange(0, D, 2, dtype=np.float32) / D)))
    freqs = pos[:, None] * inv_freq[None, :]                      # [S, 64]
    emb = np.concatenate([freqs, freqs], axis=1)                  # [S, 128]
    cosT = np.cos(emb).T.astype(np.float32)                       # [128, S]
    sinT = np.sin(emb).T.astype(np.float32)
    sinflipT = np.concatenate([-sinT[:64], sinT[64:]], axis=0).astype(np.float32)
    return np.ascontiguousarray(cosT), np.ascontiguousarray(sinflipT)


def kernel(hidden_states, position_ids, attention_mask, Wq, Wk, Wv, Wo):
    hidden_states = np.asarray(hidden_states)
    B = hidden_states.shape[0]
    assert hidden_states.shape == (B, S, E), hidden_states.shape
    assert B == 1

    mask = np.asarray(attention_mask, dtype=np.float32)[0, 0]
    if not mask.any():
        mode = "full"
    elif np.array_equal(mask, np.triu(np.full((S, S), NEG, dtype=np.float32), 1)):
        mode = "causal"
    else:
        mode = "general"

    if mode not in _PROGRAMS:
        _PROGRAMS[mode] = _build_program(mode)
    nc = _PROGRAMS[mode]

    hs = np.asarray(hidden_states[0], dtype=np.float32)
    # [E, S] -> group-major [NG, E, TG], bf16
    hsT = np.ascontiguousarray(
        hs.T.reshape(E, NG, TG).transpose(1, 0, 2)).astype(NPBF)
    cosT, sinflipT = _rope_tables(np.asarray(position_ids))
    # fold the 1/sqrt(D) score scaling into Wq so q and k share rope tables
    Wq = np.asarray(Wq, dtype=np.float32) * np.float32(1.0 / np.sqrt(D))
    Wk = np.asarray(Wk, dtype=np.float32)
    Wv = np.asarray(Wv, dtype=np.float32)
    Wo = np.asarray(Wo, dtype=np.float32)

    in_maps = []
    for c in range(NCORES):
        wqkv = np.concatenate([
            Wq[512 * c:512 * (c + 1), :].T,
            Wk[128 * c:128 * (c + 1), :].T,
            Wv[128 * c:128 * (c + 1), :].T,
        ], axis=1)
        m = {
            "hsT": hsT,
            "wqkvT": np.ascontiguousarray(wqkv).astype(NPBF),
            "woT": np.ascontiguousarray(Wo[:, 512 * c:512 * (c + 1)].T).astype(NPBF),
            "cosT": cosT, "sinT": sinflipT,
        }
        if mode == "causal":
            m["cmask"] = _causal_mask_tiles()
        elif mode == "general":
            m["maskT"] = np.ascontiguousarray(mask.T)
        in_maps.append(m)

    res = run_bass_kernel_spmd(nc, in_maps, core_ids=list(range(NCORES)),
                               trace=TRACE[0])
    LAST_EXEC_NS[0] = res.exec_time_ns
    LAST_RES[0] = res

    acc = np.zeros((NK, E // TG, 128, TG), dtype=np.float32)
    for c in range(NCORES):
        acc += res.results[c]["outp"].astype(np.float32)
    out = acc.transpose(0, 2, 1, 3).reshape(S, E)
    return out[None, :, :]
